# revision 16
# baseline (speedup 1.0000x reference)
"""ContinuousMambaLayer Trainium2 kernel.

Sharding: core c handles batch b = c>>1, sequence half hf = c&1 (1024 rows).
Each core redundantly computes the scan inputs (dm/xm/B/C) for its FULL batch
(2048 rows) so the length-2048 recurrence runs locally with no collectives,
then produces the output only for its own 1024 rows.

The host rolls each core's sequence so its own rows occupy columns [0, R).
Row-wise phases don't care about order; the scan does, so it runs in two
chained pieces: scan(cols [R, 2R)) first, whose end state (masked by the
host-supplied 0/1 `wsel`) seeds scan(cols [0, R)). For hf=0 cores the roll is
the identity and wsel=0; for hf=1 cores piece B is the true first half and
its carry feeds the own-half scan.

Layout: activations are feature-major [d on partitions (8 k-tiles), rows on
free]; host passes x/delta pre-transposed. All matmuls run in fp32r
(bit-identical to fp32 on TRN2, 1 cycle/row at N=512). Cross-partition
reductions (dm, xm, LN stats, scan y) are ones-matmuls whose [128, N] output
doubles as the partition-broadcast. The recurrence itself is
tensor_tensor_scan over [16 states, 1024 steps] x2.
"""

import numpy as np

D = 1024
KD = 8          # d / 128
S = 2048
R = 1024        # own rows per core
NST = 16        # d_state
F = 4096
KF = 32         # f / 128
NCH = S // 512  # 512-col chunks over full batch
OCH = R // 512  # 512-col chunks over own rows
RC = 512        # ffn row-chunk width

_CACHE = {}


def _build():
    import concourse.tile as tile
    from concourse import bacc, mybir

    F32 = mybir.dt.float32
    F32R = mybir.dt.float32r
    AF = mybir.ActivationFunctionType
    OP = mybir.AluOpType

    nc = bacc.Bacc(None, target_bir_lowering=False, num_swdge_queues=4)

    di = lambda n, sh, dt=F32R: nc.dram_tensor(n, sh, dt, kind="ExternalInput")
    xT_d = di("xT", [KD, 128, S])
    deT_d = di("deT", [KD, 128, S])
    w1_d = di("w1", [KD, 128, D])
    w2s_d = di("w2s", [128, KD], F32)
    tB_d = di("tBd", [NST, S], F32)
    tC_d = di("tCd", [NST, S], F32)
    xm_d = di("xmd", [NST, S], F32)
    opw_d = di("opw", [KD, 128, D])
    fw1_d = di("fw1", [KD, 128, F])
    fw2_d = di("fw2", [KF, 128, D])
    ones_d = di("ones", [128, 128])
    dpb_d = di("dpb", [128, KD], F32)
    opb_d = di("opb", [128, KD], F32)
    fb1_d = di("fb1", [128, KF], F32)
    fb2_d = di("fb2", [128, KD], F32)
    lng_d = di("lng", [128, KD], F32)
    lnb_d = di("lnb", [128, KD], F32)
    Dv_d = di("Dv", [128, KD], F32)
    xpb_d = di("xpb", [128, 1], F32)
    A16_d = di("A16", [NST, 1], F32)
    Ab1_d = di("Ab1", [NST, 1], F32)
    b2b_d = di("b2b", [NST, 1], F32)
    eps_d = di("epsv", [128, 1], F32)
    wsel_d = di("wsel", [NST, 1], F32)
    out_d = nc.dram_tensor("outT", [KD, 128, R], F32, kind="ExternalOutput")

    with tile.TileContext(nc) as tc:
        # global pools (whole kernel)
        cp = tc.alloc_tile_pool(name="consts", bufs=1, side="left")
        scn = tc.alloc_tile_pool(name="scan", bufs=1, side="left")
        wps = tc.alloc_tile_pool(name="wps", bufs=4, space="PSUM")
        f2ps = tc.alloc_tile_pool(name="f2ps", bufs=1, space="PSUM")

        # ---- constants
        ones = cp.tile([128, 128], F32R)
        nc.sync.dma_start(out=ones[:], in_=ones_d[:])
        w2s = cp.tile([128, KD], F32)
        nc.sync.dma_start(out=w2s[:], in_=w2s_d[:])
        onesb = cp.tile([128, NST], mybir.dt.bfloat16)
        nc.scalar.activation(onesb[:], ones[:, :NST].bitcast(F32), AF.Copy)
        dpb = cp.tile([128, KD], F32)
        opb = cp.tile([128, KD], F32)
        fb1 = cp.tile([128, KF], F32)
        fb2 = cp.tile([128, KD], F32)
        lng = cp.tile([128, KD], F32)
        lnb = cp.tile([128, KD], F32)
        Dv = cp.tile([128, KD], F32)
        for t, dd in ((dpb, dpb_d), (opb, opb_d), (fb1, fb1_d), (fb2, fb2_d),
                      (lng, lng_d), (lnb, lnb_d), (Dv, Dv_d)):
            nc.sync.dma_start(out=t[:], in_=dd[:])
        xpb = cp.tile([128, 1], F32)
        A16 = cp.tile([NST, 1], F32)
        Ab1 = cp.tile([NST, 1], F32)
        b2b = cp.tile([NST, 1], F32)
        eps = cp.tile([128, 1], F32)
        wsel = cp.tile([NST, 1], F32)
        carry = cp.tile([NST, 1], F32)
        for t, dd in ((xpb, xpb_d), (A16, A16_d), (Ab1, Ab1_d), (b2b, b2b_d),
                      (eps, eps_d), (wsel, wsel_d)):
            nc.sync.dma_start(out=t[:], in_=dd[:])

        # ---- scan-state tiles: DVE ops need all operands on the SAME
        # partitions, so every [16, *] item lives on partitions 0:16 as its
        # own tile; tag-shared slots recycle the column space (peak 6 live).
        ys_s = scn.tile([128, R], F32)
        mu_s = scn.tile([128, R], F32)
        rstd = scn.tile([128, R], F32)
        msq = scn.tile([128, R], F32)
        var = scn.tile([128, R], F32)
        s16 = lambda nm, w=S: scn.tile([NST, w], F32, tag="s16", bufs=6,
                                       name=nm)
        tB = s16("tB")
        tC = s16("tC")
        tXM = s16("tXM")
        tDM = s16("tDM")

        # ---- phase A pools
        c2 = tc.alloc_tile_pool(name="c2", bufs=2, side="right")
        w4 = tc.alloc_tile_pool(name="w4", bufs=1, side="right")
        h1p = tc.alloc_tile_pool(name="h1p", bufs=3, side="right")
        xop = tc.alloc_tile_pool(name="xop", bufs=2, side="left")

        # ---- B/C/xm come precomputed from the host (exact fp32; the PE
        # truncates matmul inputs to ~13-bit mantissa, and the recurrence
        # amplifies scan-input error ~500x)
        nc.sync.dma_start(out=tB[:], in_=tB_d[:])
        nc.sync.dma_start(out=tC[:], in_=tC_d[:])
        nc.sync.dma_start(out=tXM[:], in_=xm_d[:])
        # own-half x tiles (columns [0, R) after the host roll) kept for yT/res
        xtc_own = []
        for ch in range(OCH):
            xo = xop.tile([128, KD, 512], F32R, tag="xown", name=f"xo{ch}")
            for k in range(KD):
                nc.sync.dma_start(out=xo[:, k, :],
                                  in_=xT_d[k, :, ch * 512:(ch + 1) * 512])
            xtc_own.append(xo)

        # ---- delta processor: z1 = deT @ w1; h1 = softplus(z1 + b1);
        #      dm16 = w2bar^T h1 (replicated-w2bar matmul, accumulated over m)
        w1 = w4.tile([128, KD, D], F32R)
        for k in range(KD):
            nc.sync.dma_start(out=w1[:, k, :], in_=w1_d[k])
        for ch in range(NCH):
            cs = slice(ch * 512, (ch + 1) * 512)
            dtc = c2.tile([128, KD, 512], F32R, tag="c2", name=f"dtc{ch}")
            for k in range(KD):
                nc.sync.dma_start(out=dtc[:, k, :], in_=deT_d[k, :, cs])
            dm_ps = wps.tile([NST, 512], F32, tag="w", name=f"dmps{ch}")
            for m in range(KD):
                z_ps = wps.tile([128, 512], F32, tag="w", name=f"zps{ch}_{m}")
                for k in range(KD):
                    nc.tensor.matmul(out=z_ps[:],
                                     lhsT=w1[:, k, m * 128:(m + 1) * 128],
                                     rhs=dtc[:, k, :],
                                     start=(k == 0), stop=(k == KD - 1))
                e1 = h1p.tile([128, 512], F32, tag="e1", name=f"e1_{ch}_{m}")
                nc.scalar.activation(e1[:], z_ps[:], AF.Exp,
                                     bias=dpb[:, m:m + 1])
                h1f = h1p.tile([128, 512], F32, tag="h1f",
                               name=f"h1f_{ch}_{m}")
                nc.scalar.activation(h1f[:], e1[:], AF.Ln, bias=1.0)
                tw = h1p.tile([128, 512], F32, tag="tw", name=f"tw_{ch}_{m}")
                nc.vector.tensor_scalar(tw[:], h1f[:], w2s[:, m:m + 1], None,
                                        OP.mult)
                th = h1p.tile([128, 512], mybir.dt.bfloat16, tag="th",
                              name=f"th_{ch}_{m}")
                nc.scalar.activation(th[:], tw[:], AF.Copy)
                tl = h1p.tile([128, 512], F32R, tag="tl", name=f"tl_{ch}_{m}")
                nc.vector.tensor_tensor(tl[:], tw[:], th[:], OP.subtract)
                nc.tensor.matmul(out=dm_ps[:], lhsT=onesb[:], rhs=th[:],
                                 start=(m == 0), stop=False)
                nc.tensor.matmul(out=dm_ps[:], lhsT=ones[:, :NST], rhs=tl[:],
                                 start=False, stop=(m == KD - 1))
            nc.scalar.activation(tDM[:, cs], dm_ps[:], AF.Identity)

        # ---- scan inputs; two chained scans (piece B then own piece A)
        tAD = s16("tAD")
        nc.vector.tensor_scalar(tAD[:], tDM[:], A16[:], Ab1[:],
                                OP.mult, OP.add)            # 1 + A*(dm+b2bar)
        tR16 = s16("tR16")
        nc.vector.scalar_tensor_tensor(tR16[:], tDM[:], b2b[:], tXM[:],
                                       OP.add, OP.mult)      # (dm+b2bar)*xm
        tBX = s16("tBX")
        nc.vector.tensor_tensor(tBX[:], tB[:], tR16[:], OP.mult)
        tHB = s16("tHB", R)
        nc.vector.tensor_tensor_scan(tHB[:], tAD[:, R:],
                                     tBX[:, R:], 0.0, OP.mult, OP.add)
        nc.vector.tensor_scalar(carry[:], tHB[:, R - 1:R], wsel[:],
                                None, OP.mult)
        tHA = s16("tHA", R)
        nc.vector.tensor_tensor_scan(tHA[:], tAD[:, :R],
                                     tBX[:, :R], carry[:], OP.mult, OP.add)
        tHC = s16("tHC", R)
        nc.vector.tensor_tensor(tHC[:].bitcast(F32R), tHA[:],
                                tC[:, :R], OP.mult)
        for ch in range(OCH):
            cs = slice(ch * 512, (ch + 1) * 512)
            ps_ys = wps.tile([128, 512], F32, tag="w", name=f"psys{ch}")
            nc.tensor.matmul(out=ps_ys[:], lhsT=ones[:NST, :],
                             rhs=tHC[:, cs].bitcast(F32R),
                             start=True, stop=True)
            nc.scalar.activation(ys_s[:, cs], ps_ys[:], AF.Identity)

        # ---- close phase-A pools, open outproj pools
        for p in (h1p, w4, c2):
            p.release()
        r4 = tc.alloc_tile_pool(name="r4", bufs=2, side="right")
        wcol = tc.alloc_tile_pool(name="wcol", bufs=3, side="right")

        # ---- y = ys + D*x (own rows), outproj, +x residual
        yT = r4.tile([128, KD, R], F32R, tag="r4")
        for k in range(KD):
            for ch in range(OCH):
                cs = slice(ch * 512, (ch + 1) * 512)
                nc.vector.scalar_tensor_tensor(
                    yT[:, k, cs], xtc_own[ch][:, k, :].bitcast(F32),
                    Dv[:, k:k + 1], ys_s[:, cs], OP.mult, OP.add)
        res = r4.tile([128, KD, R], F32R, tag="r4")
        for m in range(KD):
            opc = wcol.tile([128, KD, 128], F32R, tag="wcol", name=f"opc{m}")
            for k in range(KD):
                nc.sync.dma_start(out=opc[:, k, :],
                                  in_=opw_d[k, :, m * 128:(m + 1) * 128])
            for ch in range(OCH):
                cs = slice(ch * 512, (ch + 1) * 512)
                o_ps = wps.tile([128, 512], F32, tag="w", name=f"ops{m}_{ch}")
                for k in range(KD):
                    nc.tensor.matmul(out=o_ps[:], lhsT=opc[:, k, :],
                                     rhs=yT[:, k, cs],
                                     start=(k == 0), stop=(k == KD - 1))
                nc.vector.scalar_tensor_tensor(
                    res[:, m, cs], o_ps[:], opb[:, m:m + 1],
                    xtc_own[ch][:, m, :].bitcast(F32), OP.add, OP.add)
        xop.release()
        sml = tc.alloc_tile_pool(name="sml", bufs=2, side="right")

        # ---- layernorm stats (ones-matmuls over partitions+tiles)
        for ch in range(OCH):
            cs = slice(ch * 512, (ch + 1) * 512)
            mu_ps = wps.tile([128, 512], F32, tag="w", name=f"mups{ch}")
            for m in range(KD):
                nc.tensor.matmul(out=mu_ps[:], lhsT=ones[:],
                                 rhs=res[:, m, cs],
                                 start=(m == 0), stop=(m == KD - 1))
            nc.scalar.activation(mu_s[:, cs], mu_ps[:],
                                 AF.Identity, scale=1.0 / D)
            ms_ps = wps.tile([128, 512], F32, tag="w", name=f"msps{ch}")
            for m in range(KD):
                sq = sml.tile([128, 512], F32R, tag="sq", name=f"sq{ch}_{m}")
                nc.scalar.activation(sq[:], res[:, m, cs].bitcast(F32),
                                     AF.Square)
                nc.tensor.matmul(out=ms_ps[:], lhsT=ones[:], rhs=sq[:],
                                 start=(m == 0), stop=(m == KD - 1))
            nc.scalar.activation(msq[:, cs], ms_ps[:],
                                 AF.Identity, scale=1.0 / D)
        nc.vector.tensor_tensor(var[:], mu_s[:], mu_s[:], OP.mult)
        nc.vector.tensor_tensor(var[:], msq[:], var[:], OP.subtract)
        nc.scalar.activation(var[:], var[:], AF.Ln, bias=eps[:])
        nc.scalar.activation(rstd[:], var[:], AF.Exp, scale=-0.5)

        normed = r4.tile([128, KD, R], F32R, tag="r4")
        for m in range(KD):
            t1 = sml.tile([128, R], F32, tag="t1", name=f"t1_{m}")
            nc.vector.tensor_tensor(t1[:], res[:, m, :].bitcast(F32), mu_s[:],
                                    OP.subtract)
            nc.vector.tensor_tensor(t1[:], t1[:], rstd[:], OP.mult)
            nc.vector.tensor_scalar(normed[:, m, :], t1[:], lng[:, m:m + 1],
                                    lnb[:, m:m + 1], OP.mult, OP.add)
        sml.release()
        scn.release()

        # ---- FFN with final residual, in row chunks of RC
        big = tc.alloc_tile_pool(name="big", bufs=1, side="right")
        wrow = tc.alloc_tile_pool(name="wrow", bufs=3, side="right")
        ot = tc.alloc_tile_pool(name="ot", bufs=3, side="right")
        for rc in range(R // RC):
            c0 = rc * RC
            hT = big.tile([128, KF, RC], F32R, tag="big", name=f"hT{rc}")
            for mf in range(KF):
                f1c = wcol.tile([128, KD, 128], F32R, tag="wcol",
                                name=f"f1c{rc}_{mf}")
                for k in range(KD):
                    nc.sync.dma_start(out=f1c[:, k, :],
                                      in_=fw1_d[k, :, mf * 128:(mf + 1) * 128])
                z_ps = wps.tile([128, RC], F32, tag="w", name=f"fzps{rc}_{mf}")
                for k in range(KD):
                    nc.tensor.matmul(out=z_ps[:], lhsT=f1c[:, k, :],
                                     rhs=normed[:, k, c0:c0 + RC],
                                     start=(k == 0), stop=(k == KD - 1))
                nc.scalar.activation(hT[:, mf, :], z_ps[:], AF.Gelu,
                                     bias=fb1[:, mf:mf + 1])
            for half in range(2):
                o_pss = [f2ps.tile([128, RC], F32, tag=f"f2_{i}",
                                   name=f"opss{rc}_{half}_{i}")
                         for i in range(4)]
                for kf in range(KF):
                    w2r = wrow.tile([128, 512], F32R, tag="wrow",
                                    name=f"w2r{rc}_{half}_{kf}")
                    nc.sync.dma_start(
                        out=w2r[:],
                        in_=fw2_d[kf, :, half * 512:(half + 1) * 512])
                    for mi in range(4):
                        nc.tensor.matmul(out=o_pss[mi][:],
                                         lhsT=w2r[:, mi * 128:(mi + 1) * 128],
                                         rhs=hT[:, kf, :],
                                         start=(kf == 0), stop=(kf == KF - 1))
                for mi in range(4):
                    m = half * 4 + mi
                    oev = ot.tile([128, RC], F32, tag="ot",
                                  name=f"oev{rc}_{half}_{mi}")
                    nc.vector.scalar_tensor_tensor(
                        oev[:], o_pss[mi][:], fb2[:, m:m + 1],
                        normed[:, m, c0:c0 + RC].bitcast(F32),
                        OP.add, OP.add)
                    nc.sync.dma_start(out=out_d[m, :, c0:c0 + RC], in_=oev[:])
        for p in (ot, wrow, big, wcol, r4, cp, f2ps, wps):
            p.release()

    nc.compile()
    return nc


def _prepare(inputs):
    """Build the 8 per-core input maps (all host-side numpy)."""
    f32 = np.float32
    x = np.asarray(inputs["input_embedding"], f32)
    de = np.asarray(inputs["delta_embedding"], f32)
    A_log = np.asarray(inputs["A_log"], f32)
    Dvec = np.asarray(inputs["D"], f32)
    xproj_w = np.asarray(inputs["xproj_w"], f32)
    xproj_b = np.asarray(inputs["xproj_b"], f32)
    outproj_w = np.asarray(inputs["outproj_w"], f32)
    outproj_b = np.asarray(inputs["outproj_b"], f32)
    dp_w1 = np.asarray(inputs["dp_w1"], f32)
    dp_b1 = np.asarray(inputs["dp_b1"], f32)
    dp_w2 = np.asarray(inputs["dp_w2"], f32)
    dp_b2 = np.asarray(inputs["dp_b2"], f32)
    ln_g = np.asarray(inputs["ln_g"], f32)
    ln_b = np.asarray(inputs["ln_b"], f32)
    ffn_w1 = np.asarray(inputs["ffn_w1"], f32)
    ffn_b1 = np.asarray(inputs["ffn_b1"], f32)
    ffn_w2 = np.asarray(inputs["ffn_w2"], f32)
    ffn_b2 = np.asarray(inputs["ffn_b2"], f32)

    w2bar = dp_w2.mean(axis=1)                       # [d]
    b2bar = f32(dp_b2.mean())
    A = (-np.exp(A_log)).astype(f32)                 # [n]

    kt = lambda w: np.ascontiguousarray(w.reshape(KD, 128, -1))

    def _xpb128(xb):
        v = np.zeros((128, 1), np.float32)
        v[0:NST, 0] = xb[:NST]
        v[32:48, 0] = xb[NST:]
        return v
    colmaj = lambda v, k: np.ascontiguousarray(v.reshape(k, 128).T)

    shared = {
        "w1": kt(dp_w1),
        "w2s": colmaj(w2bar, KD),
        "opw": kt(outproj_w),
        "fw1": kt(ffn_w1),
        "fw2": kt(ffn_w2),
        "ones": np.ones((128, 128), f32),
        "dpb": colmaj(dp_b1, KD),
        "opb": colmaj(outproj_b, KD),
        "fb1": colmaj(ffn_b1, KF),
        "fb2": colmaj(ffn_b2, KD),
        "lng": colmaj(ln_g, KD),
        "lnb": colmaj(ln_b, KD),
        "Dv": colmaj(Dvec, KD),
        "xpb": _xpb128(xproj_b),
        "A16": np.ascontiguousarray(A.reshape(NST, 1)),
        "Ab1": np.ascontiguousarray(
            (A * b2bar + 1.0).astype(f32).reshape(NST, 1)),
        "b2b": np.full((NST, 1), b2bar, f32),
        "epsv": np.full((128, 1), 1e-5, f32),
    }
    maps = []
    for c in range(8):
        b, hf = c >> 1, c & 1
        xr = np.roll(x[b], -hf * R, axis=0)
        dr = np.roll(de[b], -hf * R, axis=0)
        m = dict(shared)
        m["xT"] = np.ascontiguousarray(xr.T).reshape(KD, 128, S)
        m["deT"] = np.ascontiguousarray(dr.T).reshape(KD, 128, S)
        m["wsel"] = np.full((NST, 1), float(hf), f32)
        bc = xr @ xproj_w + xproj_b                    # exact host fp32
        m["tBd"] = np.ascontiguousarray(bc[:, :NST].T)
        m["tCd"] = np.ascontiguousarray(bc[:, NST:].T)
        m["xmd"] = np.ascontiguousarray(
            np.broadcast_to(xr.mean(-1, dtype=f32)[None, :], (NST, S)))
        maps.append(m)
    return maps


def kernel(**inputs):
    from concourse.bass_utils import run_bass_kernel_spmd

    if "nc" not in _CACHE:
        _CACHE["nc"] = _build()
    nc = _CACHE["nc"]
    maps = _prepare(inputs)
    res = run_bass_kernel_spmd(nc, maps, list(range(8))).results
    out = np.empty((4, S, D), np.float32)
    for c in range(8):
        b, hf = c >> 1, c & 1
        arr = res[c]["outT"].reshape(D, R)       # [features, own rows]
        out[b, hf * R:(hf + 1) * R, :] = arr.T
    return out


# revision 18
# speedup vs baseline: 1.0021x; 1.0021x over previous
"""ContinuousMambaLayer Trainium2 kernel.

Sharding: core c handles batch b = c>>1, sequence half hf = c&1 (1024 rows).
Each core redundantly computes the scan inputs (dm/xm/B/C) for its FULL batch
(2048 rows) so the length-2048 recurrence runs locally with no collectives,
then produces the output only for its own 1024 rows.

The host rolls each core's sequence so its own rows occupy columns [0, R).
Row-wise phases don't care about order; the scan does, so it runs in two
chained pieces: scan(cols [R, 2R)) first, whose end state (masked by the
host-supplied 0/1 `wsel`) seeds scan(cols [0, R)). For hf=0 cores the roll is
the identity and wsel=0; for hf=1 cores piece B is the true first half and
its carry feeds the own-half scan.

Layout: activations are feature-major [d on partitions (8 k-tiles), rows on
free]; host passes x/delta pre-transposed. All matmuls run in fp32r
(bit-identical to fp32 on TRN2, 1 cycle/row at N=512). Cross-partition
reductions (dm, xm, LN stats, scan y) are ones-matmuls whose [128, N] output
doubles as the partition-broadcast. The recurrence itself is
tensor_tensor_scan over [16 states, 1024 steps] x2.
"""

import numpy as np

D = 1024
KD = 8          # d / 128
S = 2048
R = 1024        # own rows per core
NST = 16        # d_state
F = 4096
KF = 32         # f / 128
NCH = S // 512  # 512-col chunks over full batch
OCH = R // 512  # 512-col chunks over own rows
RC = 512        # ffn row-chunk width

_CACHE = {}


def _build():
    import concourse.tile as tile
    from concourse import bacc, mybir

    F32 = mybir.dt.float32
    F32R = mybir.dt.float32r
    AF = mybir.ActivationFunctionType
    OP = mybir.AluOpType

    nc = bacc.Bacc(None, target_bir_lowering=False, num_swdge_queues=4)

    di = lambda n, sh, dt=F32R: nc.dram_tensor(n, sh, dt, kind="ExternalInput")
    xT_d = di("xT", [KD, 128, S])
    deT_d = di("deT", [KD, 128, S])
    w1_d = di("w1", [KD, 128, D])
    w2s_d = di("w2s", [128, KD], F32)
    tB_d = di("tBd", [NST, S], F32)
    tC_d = di("tCd", [NST, S], F32)
    xm_d = di("xmd", [NST, S], F32)
    opw_d = di("opw", [KD, 128, D])
    fw1_d = di("fw1", [KD, 128, F])
    fw2_d = di("fw2", [KF, 128, D])
    ones_d = di("ones", [128, 128])
    dpb_d = di("dpb", [128, KD], F32)
    opb_d = di("opb", [128, KD], F32)
    fb1_d = di("fb1", [128, KF], F32)
    fb2_d = di("fb2", [128, KD], F32)
    lng_d = di("lng", [128, KD], F32)
    lnb_d = di("lnb", [128, KD], F32)
    Dv_d = di("Dv", [128, KD], F32)
    xpb_d = di("xpb", [128, 1], F32)
    A16_d = di("A16", [NST, 1], F32)
    Ab1_d = di("Ab1", [NST, 1], F32)
    b2b_d = di("b2b", [NST, 1], F32)
    eps_d = di("epsv", [128, 1], F32)
    wsel_d = di("wsel", [NST, 1], F32)
    out_d = nc.dram_tensor("outT", [KD, 128, R], F32, kind="ExternalOutput")

    with tile.TileContext(nc) as tc:
        # global pools (whole kernel)
        cp = tc.alloc_tile_pool(name="consts", bufs=1, side="left")
        scn = tc.alloc_tile_pool(name="scan", bufs=1, side="left")
        wps = tc.alloc_tile_pool(name="wps", bufs=4, space="PSUM")
        f2ps = tc.alloc_tile_pool(name="f2ps", bufs=1, space="PSUM")

        # ---- constants
        ones = cp.tile([128, 128], F32R)
        nc.sync.dma_start(out=ones[:], in_=ones_d[:])
        w2s = cp.tile([128, KD], F32)
        nc.sync.dma_start(out=w2s[:], in_=w2s_d[:])
        onesb = cp.tile([128, NST], mybir.dt.bfloat16)
        nc.scalar.activation(onesb[:], ones[:, :NST].bitcast(F32), AF.Copy)
        dpb = cp.tile([128, KD], F32)
        opb = cp.tile([128, KD], F32)
        fb1 = cp.tile([128, KF], F32)
        fb2 = cp.tile([128, KD], F32)
        lng = cp.tile([128, KD], F32)
        lnb = cp.tile([128, KD], F32)
        Dv = cp.tile([128, KD], F32)
        for t, dd in ((dpb, dpb_d), (opb, opb_d), (fb1, fb1_d), (fb2, fb2_d),
                      (lng, lng_d), (lnb, lnb_d), (Dv, Dv_d)):
            nc.sync.dma_start(out=t[:], in_=dd[:])
        xpb = cp.tile([128, 1], F32)
        A16 = cp.tile([NST, 1], F32)
        Ab1 = cp.tile([NST, 1], F32)
        b2b = cp.tile([NST, 1], F32)
        eps = cp.tile([128, 1], F32)
        wsel = cp.tile([NST, 1], F32)
        carry = cp.tile([NST, 1], F32)
        for t, dd in ((xpb, xpb_d), (A16, A16_d), (Ab1, Ab1_d), (b2b, b2b_d),
                      (eps, eps_d), (wsel, wsel_d)):
            nc.sync.dma_start(out=t[:], in_=dd[:])

        # ---- scan-state tiles: DVE ops need all operands on the SAME
        # partitions, so every [16, *] item lives on partitions 0:16 as its
        # own tile; tag-shared slots recycle the column space (peak 6 live).
        ys_s = scn.tile([128, R], F32)
        mu_s = scn.tile([128, R], F32)
        rstd = scn.tile([128, R], F32)
        msq = scn.tile([128, R], F32)
        var = scn.tile([128, R], F32)
        s16 = lambda nm, w=S: scn.tile([NST, w], F32, tag="s16", bufs=6,
                                       name=nm)
        tB = s16("tB")
        tC = s16("tC")
        tXM = s16("tXM")
        tDM = s16("tDM")

        # ---- phase A pools
        c2 = tc.alloc_tile_pool(name="c2", bufs=2, side="right")
        w4 = tc.alloc_tile_pool(name="w4", bufs=1, side="right")
        h1p = tc.alloc_tile_pool(name="h1p", bufs=4, side="right")
        xop = tc.alloc_tile_pool(name="xop", bufs=2, side="left")

        # ---- B/C/xm come precomputed from the host (exact fp32; the PE
        # truncates matmul inputs to ~13-bit mantissa, and the recurrence
        # amplifies scan-input error ~500x)
        nc.sync.dma_start(out=tB[:], in_=tB_d[:])
        nc.sync.dma_start(out=tC[:], in_=tC_d[:])
        nc.sync.dma_start(out=tXM[:], in_=xm_d[:])
        # own-half x tiles (columns [0, R) after the host roll) kept for yT/res
        xtc_own = []
        for ch in range(OCH):
            xo = xop.tile([128, KD, 512], F32R, tag="xown", name=f"xo{ch}")
            for k in range(KD):
                nc.sync.dma_start(out=xo[:, k, :],
                                  in_=xT_d[k, :, ch * 512:(ch + 1) * 512])
            xtc_own.append(xo)

        # ---- delta processor: z1 = deT @ w1; h1 = softplus(z1 + b1);
        #      dm16 = w2bar^T h1 (replicated-w2bar matmul, accumulated over m)
        w1 = w4.tile([128, KD, D], F32R)
        for k in range(KD):
            nc.sync.dma_start(out=w1[:, k, :], in_=w1_d[k])
        for ch in range(NCH):
            cs = slice(ch * 512, (ch + 1) * 512)
            dtc = c2.tile([128, KD, 512], F32R, tag="c2", name=f"dtc{ch}")
            for k in range(KD):
                nc.sync.dma_start(out=dtc[:, k, :], in_=deT_d[k, :, cs])
            dm_ps = wps.tile([NST, 512], F32, tag="w", name=f"dmps{ch}")
            for m in range(KD):
                z_ps = wps.tile([128, 512], F32, tag="w", name=f"zps{ch}_{m}")
                for k in range(KD):
                    nc.tensor.matmul(out=z_ps[:],
                                     lhsT=w1[:, k, m * 128:(m + 1) * 128],
                                     rhs=dtc[:, k, :],
                                     start=(k == 0), stop=(k == KD - 1))
                e1 = h1p.tile([128, 512], F32, tag="e1", name=f"e1_{ch}_{m}")
                nc.scalar.activation(e1[:], z_ps[:], AF.Exp,
                                     bias=dpb[:, m:m + 1])
                h1f = h1p.tile([128, 512], F32, tag="h1f",
                               name=f"h1f_{ch}_{m}")
                nc.scalar.activation(h1f[:], e1[:], AF.Ln, bias=1.0)
                tw = h1p.tile([128, 512], F32, tag="tw", name=f"tw_{ch}_{m}")
                nc.vector.tensor_scalar(tw[:], h1f[:], w2s[:, m:m + 1], None,
                                        OP.mult)
                th = h1p.tile([128, 512], mybir.dt.bfloat16, tag="th",
                              name=f"th_{ch}_{m}")
                nc.scalar.activation(th[:], tw[:], AF.Copy)
                tl = h1p.tile([128, 512], F32R, tag="tl", name=f"tl_{ch}_{m}")
                nc.vector.tensor_tensor(tl[:], tw[:], th[:], OP.subtract)
                nc.tensor.matmul(out=dm_ps[:], lhsT=onesb[:], rhs=th[:],
                                 start=(m == 0), stop=False)
                nc.tensor.matmul(out=dm_ps[:], lhsT=ones[:, :NST], rhs=tl[:],
                                 start=False, stop=(m == KD - 1))
            nc.scalar.activation(tDM[:, cs], dm_ps[:], AF.Identity)

        # ---- scan inputs; two chained scans (piece B then own piece A)
        tAD = s16("tAD")
        nc.vector.tensor_scalar(tAD[:], tDM[:], A16[:], Ab1[:],
                                OP.mult, OP.add)            # 1 + A*(dm+b2bar)
        tR16 = s16("tR16")
        nc.vector.scalar_tensor_tensor(tR16[:], tDM[:], b2b[:], tXM[:],
                                       OP.add, OP.mult)      # (dm+b2bar)*xm
        tBX = s16("tBX")
        nc.vector.tensor_tensor(tBX[:], tB[:], tR16[:], OP.mult)
        tHB = s16("tHB", R)
        nc.vector.tensor_tensor_scan(tHB[:], tAD[:, R:],
                                     tBX[:, R:], 0.0, OP.mult, OP.add)
        nc.vector.tensor_scalar(carry[:], tHB[:, R - 1:R], wsel[:],
                                None, OP.mult)
        tHA = s16("tHA", R)
        nc.vector.tensor_tensor_scan(tHA[:], tAD[:, :R],
                                     tBX[:, :R], carry[:], OP.mult, OP.add)
        tHC = s16("tHC", R)
        nc.vector.tensor_tensor(tHC[:].bitcast(F32R), tHA[:],
                                tC[:, :R], OP.mult)
        for ch in range(OCH):
            cs = slice(ch * 512, (ch + 1) * 512)
            ps_ys = wps.tile([128, 512], F32, tag="w", name=f"psys{ch}")
            nc.tensor.matmul(out=ps_ys[:], lhsT=ones[:NST, :],
                             rhs=tHC[:, cs].bitcast(F32R),
                             start=True, stop=True)
            nc.scalar.activation(ys_s[:, cs], ps_ys[:], AF.Identity)

        # ---- close phase-A pools, open outproj pools
        for p in (h1p, w4, c2):
            p.release()
        r4 = tc.alloc_tile_pool(name="r4", bufs=2, side="right")
        wcol = tc.alloc_tile_pool(name="wcol", bufs=3, side="right")

        # ---- y = ys + D*x (own rows), outproj, +x residual
        yT = r4.tile([128, KD, R], F32R, tag="r4")
        for k in range(KD):
            for ch in range(OCH):
                cs = slice(ch * 512, (ch + 1) * 512)
                nc.vector.scalar_tensor_tensor(
                    yT[:, k, cs], xtc_own[ch][:, k, :].bitcast(F32),
                    Dv[:, k:k + 1], ys_s[:, cs], OP.mult, OP.add)
        res = r4.tile([128, KD, R], F32R, tag="r4")
        for m in range(KD):
            opc = wcol.tile([128, KD, 128], F32R, tag="wcol", name=f"opc{m}")
            for k in range(KD):
                nc.sync.dma_start(out=opc[:, k, :],
                                  in_=opw_d[k, :, m * 128:(m + 1) * 128])
            for ch in range(OCH):
                cs = slice(ch * 512, (ch + 1) * 512)
                o_ps = wps.tile([128, 512], F32, tag="w", name=f"ops{m}_{ch}")
                for k in range(KD):
                    nc.tensor.matmul(out=o_ps[:], lhsT=opc[:, k, :],
                                     rhs=yT[:, k, cs],
                                     start=(k == 0), stop=(k == KD - 1))
                nc.vector.scalar_tensor_tensor(
                    res[:, m, cs], o_ps[:], opb[:, m:m + 1],
                    xtc_own[ch][:, m, :].bitcast(F32), OP.add, OP.add)
        xop.release()
        sml = tc.alloc_tile_pool(name="sml", bufs=2, side="right")

        # ---- layernorm stats (ones-matmuls over partitions+tiles)
        for ch in range(OCH):
            cs = slice(ch * 512, (ch + 1) * 512)
            mu_ps = wps.tile([128, 512], F32, tag="w", name=f"mups{ch}")
            for m in range(KD):
                nc.tensor.matmul(out=mu_ps[:], lhsT=ones[:],
                                 rhs=res[:, m, cs],
                                 start=(m == 0), stop=(m == KD - 1))
            nc.scalar.activation(mu_s[:, cs], mu_ps[:],
                                 AF.Identity, scale=1.0 / D)
            ms_ps = wps.tile([128, 512], F32, tag="w", name=f"msps{ch}")
            for m in range(KD):
                sq = sml.tile([128, 512], F32R, tag="sq", name=f"sq{ch}_{m}")
                nc.scalar.activation(sq[:], res[:, m, cs].bitcast(F32),
                                     AF.Square)
                nc.tensor.matmul(out=ms_ps[:], lhsT=ones[:], rhs=sq[:],
                                 start=(m == 0), stop=(m == KD - 1))
            nc.scalar.activation(msq[:, cs], ms_ps[:],
                                 AF.Identity, scale=1.0 / D)
        nc.vector.tensor_tensor(var[:], mu_s[:], mu_s[:], OP.mult)
        nc.vector.tensor_tensor(var[:], msq[:], var[:], OP.subtract)
        nc.scalar.activation(var[:], var[:], AF.Ln, bias=eps[:])
        nc.scalar.activation(rstd[:], var[:], AF.Exp, scale=-0.5)

        normed = r4.tile([128, KD, R], F32R, tag="r4")
        for m in range(KD):
            t1 = sml.tile([128, R], F32, tag="t1", name=f"t1_{m}")
            nc.vector.tensor_tensor(t1[:], res[:, m, :].bitcast(F32), mu_s[:],
                                    OP.subtract)
            nc.vector.tensor_tensor(t1[:], t1[:], rstd[:], OP.mult)
            nc.vector.tensor_scalar(normed[:, m, :], t1[:], lng[:, m:m + 1],
                                    lnb[:, m:m + 1], OP.mult, OP.add)
        sml.release()
        scn.release()

        # ---- FFN with final residual, in row chunks of RC
        big = tc.alloc_tile_pool(name="big", bufs=1, side="right")
        wrow = tc.alloc_tile_pool(name="wrow", bufs=3, side="right")
        ot = tc.alloc_tile_pool(name="ot", bufs=4, side="right")
        for rc in range(R // RC):
            c0 = rc * RC
            hT = big.tile([128, KF, RC], F32R, tag="big", name=f"hT{rc}")
            for mf in range(KF):
                f1c = wcol.tile([128, KD, 128], F32R, tag="wcol",
                                name=f"f1c{rc}_{mf}")
                for k in range(KD):
                    nc.sync.dma_start(out=f1c[:, k, :],
                                      in_=fw1_d[k, :, mf * 128:(mf + 1) * 128])
                z_ps = wps.tile([128, RC], F32, tag="w", name=f"fzps{rc}_{mf}")
                for k in range(KD):
                    nc.tensor.matmul(out=z_ps[:], lhsT=f1c[:, k, :],
                                     rhs=normed[:, k, c0:c0 + RC],
                                     start=(k == 0), stop=(k == KD - 1))
                nc.scalar.activation(hT[:, mf, :], z_ps[:], AF.Gelu,
                                     bias=fb1[:, mf:mf + 1])
            for half in range(2):
                o_pss = [f2ps.tile([128, RC], F32, tag=f"f2_{i}",
                                   name=f"opss{rc}_{half}_{i}")
                         for i in range(4)]
                for kf in range(KF):
                    w2r = wrow.tile([128, 512], F32R, tag="wrow",
                                    name=f"w2r{rc}_{half}_{kf}")
                    nc.sync.dma_start(
                        out=w2r[:],
                        in_=fw2_d[kf, :, half * 512:(half + 1) * 512])
                    for mi in range(4):
                        nc.tensor.matmul(out=o_pss[mi][:],
                                         lhsT=w2r[:, mi * 128:(mi + 1) * 128],
                                         rhs=hT[:, kf, :],
                                         start=(kf == 0), stop=(kf == KF - 1))
                for mi in range(4):
                    m = half * 4 + mi
                    oev = ot.tile([128, RC], F32, tag="ot",
                                  name=f"oev{rc}_{half}_{mi}")
                    nc.vector.scalar_tensor_tensor(
                        oev[:], o_pss[mi][:], fb2[:, m:m + 1],
                        normed[:, m, c0:c0 + RC].bitcast(F32),
                        OP.add, OP.add)
                    nc.sync.dma_start(out=out_d[m, :, c0:c0 + RC], in_=oev[:])
        for p in (ot, wrow, big, wcol, r4, cp, f2ps, wps):
            p.release()

    nc.compile()
    return nc


def _prepare(inputs):
    """Build the 8 per-core input maps (all host-side numpy)."""
    f32 = np.float32
    x = np.asarray(inputs["input_embedding"], f32)
    de = np.asarray(inputs["delta_embedding"], f32)
    A_log = np.asarray(inputs["A_log"], f32)
    Dvec = np.asarray(inputs["D"], f32)
    xproj_w = np.asarray(inputs["xproj_w"], f32)
    xproj_b = np.asarray(inputs["xproj_b"], f32)
    outproj_w = np.asarray(inputs["outproj_w"], f32)
    outproj_b = np.asarray(inputs["outproj_b"], f32)
    dp_w1 = np.asarray(inputs["dp_w1"], f32)
    dp_b1 = np.asarray(inputs["dp_b1"], f32)
    dp_w2 = np.asarray(inputs["dp_w2"], f32)
    dp_b2 = np.asarray(inputs["dp_b2"], f32)
    ln_g = np.asarray(inputs["ln_g"], f32)
    ln_b = np.asarray(inputs["ln_b"], f32)
    ffn_w1 = np.asarray(inputs["ffn_w1"], f32)
    ffn_b1 = np.asarray(inputs["ffn_b1"], f32)
    ffn_w2 = np.asarray(inputs["ffn_w2"], f32)
    ffn_b2 = np.asarray(inputs["ffn_b2"], f32)

    w2bar = dp_w2.mean(axis=1)                       # [d]
    b2bar = f32(dp_b2.mean())
    A = (-np.exp(A_log)).astype(f32)                 # [n]

    kt = lambda w: np.ascontiguousarray(w.reshape(KD, 128, -1))

    def _xpb128(xb):
        v = np.zeros((128, 1), np.float32)
        v[0:NST, 0] = xb[:NST]
        v[32:48, 0] = xb[NST:]
        return v
    colmaj = lambda v, k: np.ascontiguousarray(v.reshape(k, 128).T)

    shared = {
        "w1": kt(dp_w1),
        "w2s": colmaj(w2bar, KD),
        "opw": kt(outproj_w),
        "fw1": kt(ffn_w1),
        "fw2": kt(ffn_w2),
        "ones": np.ones((128, 128), f32),
        "dpb": colmaj(dp_b1, KD),
        "opb": colmaj(outproj_b, KD),
        "fb1": colmaj(ffn_b1, KF),
        "fb2": colmaj(ffn_b2, KD),
        "lng": colmaj(ln_g, KD),
        "lnb": colmaj(ln_b, KD),
        "Dv": colmaj(Dvec, KD),
        "xpb": _xpb128(xproj_b),
        "A16": np.ascontiguousarray(A.reshape(NST, 1)),
        "Ab1": np.ascontiguousarray(
            (A * b2bar + 1.0).astype(f32).reshape(NST, 1)),
        "b2b": np.full((NST, 1), b2bar, f32),
        "epsv": np.full((128, 1), 1e-5, f32),
    }
    maps = []
    for c in range(8):
        b, hf = c >> 1, c & 1
        xr = np.roll(x[b], -hf * R, axis=0)
        dr = np.roll(de[b], -hf * R, axis=0)
        m = dict(shared)
        m["xT"] = np.ascontiguousarray(xr.T).reshape(KD, 128, S)
        m["deT"] = np.ascontiguousarray(dr.T).reshape(KD, 128, S)
        m["wsel"] = np.full((NST, 1), float(hf), f32)
        bc = xr @ xproj_w + xproj_b                    # exact host fp32
        m["tBd"] = np.ascontiguousarray(bc[:, :NST].T)
        m["tCd"] = np.ascontiguousarray(bc[:, NST:].T)
        m["xmd"] = np.ascontiguousarray(
            np.broadcast_to(xr.mean(-1, dtype=f32)[None, :], (NST, S)))
        maps.append(m)
    return maps


def kernel(**inputs):
    from concourse.bass_utils import run_bass_kernel_spmd

    if "nc" not in _CACHE:
        _CACHE["nc"] = _build()
    nc = _CACHE["nc"]
    maps = _prepare(inputs)
    res = run_bass_kernel_spmd(nc, maps, list(range(8))).results
    out = np.empty((4, S, D), np.float32)
    for c in range(8):
        b, hf = c >> 1, c & 1
        arr = res[c]["outT"].reshape(D, R)       # [features, own rows]
        out[b, hf * R:(hf + 1) * R, :] = arr.T
    return out
